# revision 22
# baseline (speedup 1.0000x reference)
"""Trainium2 Bass kernel for nn_DecoderGRUWeighted (batch-1 GRU decoder step).

Strategy (8 NeuronCores, SPMD):
  - Vocab dim of the output projection Wo (50257x1024, the dominant memory
    traffic) is sharded 8 ways; each core computes a [1, V/8] logits slice
    with a 4-way column-tiled PE matmul (4 concurrent streams).
  - Attention scores are sharded by L and combined with one tiny AllGather;
    each core then computes the full softmax + context locally (encoder
    replicated). GRU gate partials are combined with one AllReduce.
  - A dependency-free dummy AllGather fires at t=0 so the cross-core
    entry barrier + first-collective setup overlap the weight streaming.
  - All length-D vectors live on chip in "column layout": SBUF tile
    [128, D/128] with element d at (partition d%128, column d//128), so every
    matvec uses natural [128,128] weight tiles as the PE stationary operand
    and [128,1] vector columns as the moving operand, with no transposes.
  - log_softmax is computed without max subtraction (logits are O(1) here):
    out = logits - ln(sum_exp), with the global sum reduced via AllGather.
  - Wo^T is stored bf16 on chip to halve HBM traffic; all the small
    weights stay f32.
"""

import sys

if "/opt/trn_rl_repo" not in sys.path:
    sys.path.insert(0, "/opt/trn_rl_repo")

import numpy as np
import ml_dtypes

H = 1024
V = 50257
L = 512
NCORES = 8
V_SH = 6283          # ceil(V / 8); global pad = 50264 (7 zero rows on core 7)
V_PAD = V_SH * NCORES
CH = 512             # logits chunk (one PSUM bank of f32)
N_CH = 13            # 12*512 + 139
G_COLS = 1675        # per-group row length: 3*512 + 139 (group 0 holds the tail)

N_FILL = 20         # PE-warming filler matmuls during the collective wait
_BF16 = np.float16

_cache: dict = {}


def _build():
    import concourse.bacc as bacc
    import concourse.tile as tile
    from concourse import mybir

    f32 = mybir.dt.float32
    bf16 = mybir.dt.float16
    AF = mybir.ActivationFunctionType

    nc = bacc.Bacc("TRN2", target_bir_lowering=False, debug=False,
                   num_devices=NCORES)

    # ---- I/O ------------------------------------------------------------
    # vecs columns: 0:16 cat1 | 16:24 embed | 24:32 hidden | 32 ones |
    #               33:49 bih+bhh (r,z) | 49:57 bih_n | 57:65 bhh_n |
    #               65 bw shard (partitions 0:64) | 66 bc shard
    vecs = nc.dram_tensor("vecs", [128, 68], f32, kind="ExternalInput")
    aux_row = nc.dram_tensor("aux_row", [1, 128], f32, kind="ExternalInput")
    wwt = nc.dram_tensor("wwt", [2048, 64], f32, kind="ExternalInput")
    enc = nc.dram_tensor("enc", [512, 1024], bf16, kind="ExternalInput")
    wct = nc.dram_tensor("wct", [2048, 128], bf16, kind="ExternalInput")
    wiht = nc.dram_tensor("wiht", [128, 3072], bf16, kind="ExternalInput")
    whht = nc.dram_tensor("whht", [128, 3072], bf16, kind="ExternalInput")
    hloc = nc.dram_tensor("hloc", [128, 1], bf16, kind="ExternalInput")
    wot = nc.dram_tensor("wot", [1024, V_SH], bf16, kind="ExternalInput")
    bo4 = nc.dram_tensor("bo4", [128, G_COLS], f32, kind="ExternalInput")

    out_sl = nc.dram_tensor("out_sl", [1, V_SH], f32, kind="ExternalOutput")
    hnew_o = nc.dram_tensor("hnew_o", [128, 8], f32, kind="ExternalOutput")
    w_o = nc.dram_tensor("w_o", [1, 512], f32, kind="ExternalOutput")

    RG = [list(range(NCORES))]

    with tile.TileContext(nc) as tc:
        with (
            tc.tile_pool(name="const", bufs=1) as cpool,
            tc.tile_pool(name="work", bufs=2) as wpool,
            tc.tile_pool(name="lgp", bufs=4, space="PSUM") as lg_pool,
            tc.tile_pool(name="spp", bufs=2, space="PSUM") as sp_pool,
            tc.tile_pool(name="dram", bufs=1, space="DRAM") as dpool,
        ):
            zz = cpool.tile([1, 8], f32)
            nc.vector.memset(zz[:], 0.0)
            ones32_bf = cpool.tile([128, 32], bf16)
            nc.vector.memset(ones32_bf[:], 1.0)
            ones128 = cpool.tile([128, 128], f32)
            nc.vector.memset(ones128[:], 1.0)

            # ---- stage the small inputs (chain-critical DMAs first) -----
            vecs_sb = cpool.tile([128, 68], f32)
            nc.sync.dma_start(vecs_sb[:], vecs.ap())
            aux_sb = cpool.tile([1, 128], f32)
            nc.sync.dma_start(aux_sb[:], aux_row.ap())
            wwt_sb = cpool.tile([128, 16, 64], f32)
            nc.sync.dma_start(
                wwt_sb[:], wwt.ap().rearrange("(c p) f -> p c f", p=128))
            hloc_sb = cpool.tile([128, 1], bf16)
            nc.sync.dma_start(hloc_sb[:], hloc.ap())
            ones_col = vecs_sb[:, 32:33]

            # bf16 copy of the embedding columns (feeds bf16 Wc matmul)
            emb_bf = cpool.tile([128, 8], bf16)
            nc.vector.tensor_copy(emb_bf[:], vecs_sb[:, 16:24])

            # ---- attention scores s = Ww_sh @ cat1 + bw_sh  ([64,1]) ----
            s_ps = sp_pool.tile([64, 1], f32, tag="sp")
            for c in range(16):
                nc.tensor.matmul(s_ps[:], wwt_sb[:, c, :],
                                 vecs_sb[:, c:c + 1],
                                 start=(c == 0), stop=(c == 15))
            s_sb = wpool.tile([64, 1], f32, tag="s_sb")
            nc.scalar.activation(s_sb[:], s_ps[:], AF.Identity,
                                 bias=vecs_sb[0:64, 65:66])

            # ---- AllGather #1: local scores -> all 512 scores -----------
            cc1_in = dpool.tile([1, 64], f32)
            cc1_out = dpool.tile([8, 64], f32)
            nc.sync.dma_start(cc1_in[:], s_sb[:])
            nc.gpsimd.collective_compute(
                "AllGather", mybir.AluOpType.bypass, replica_groups=RG,
                ins=[cc1_in.opt()], outs=[cc1_out.opt()])

            enc_sb = cpool.tile([128, 4, 1024], bf16)
            nc.sync.dma_start(
                enc_sb[:], enc.ap().rearrange("(c p) f -> p c f", p=128))
            wct_sb = cpool.tile([128, 16, 128], bf16)
            nc.sync.dma_start(
                wct_sb[:], wct.ap().rearrange("(c p) f -> p c f", p=128))
            wiht_sb = cpool.tile([128, 3072], bf16)
            nc.sync.dma_start(wiht_sb[:], wiht.ap())
            whht_sb = cpool.tile([128, 3072], bf16)
            nc.sync.dma_start(whht_sb[:], whht.ap())
            bo4_sb = cpool.tile([128, G_COLS], f32)
            nc.sync.dma_start(bo4_sb[:], bo4.ap())

            # gh partials depend only on inputs: run during the AllGather
            # wait (useful work + keeps the PE array warm)
            ghh_ps = sp_pool.tile([128, 48], f32, tag="sp")
            for m in range(24):
                nc.tensor.matmul(ghh_ps[:, m:m + 1],
                                 whht_sb[:, m * 128:(m + 1) * 128],
                                 hloc_sb[:], start=True, stop=True)

            # ---- big Wo^T shard: 8 contraction tiles, streamed early ----
            wot_sb = cpool.tile([128, 8, V_SH], bf16)
            for k in range(8):
                nc.sync.dma_start(
                    wot_sb[:, k, :], wot.ap()[k * 128:(k + 1) * 128, :])

            # readback into column layout [128, 4]: element l=128t+64e+j at
            # (partition 64e+j, col t); gathered row r=2t+e holds j=0..63
            scores_col = wpool.tile([128, 4], f32, tag="scores_col")
            cc1_v = cc1_out.opt().rearrange("(t e) j -> e j t", e=2)
            nc.gpsimd.dma_start(scores_col[0:64, 0:4], cc1_v[0:1])
            nc.gpsimd.dma_start(scores_col[64:128, 0:4], cc1_v[1:2])

            # full softmax (local): w = exp(s) / sum(exp(s))
            exp4 = wpool.tile([128, 4], f32, tag="exp4")
            acc4 = wpool.tile([128, 1], f32, tag="acc4")
            nc.scalar.activation(exp4[:], scores_col[:], AF.Exp,
                                 accum_out=acc4[:])
            S_ps = sp_pool.tile([128, 1], f32, tag="sp")
            nc.tensor.matmul(S_ps[:], ones128[:], acc4[:],
                             start=True, stop=True)
            S128 = wpool.tile([128, 1], f32, tag="S128")
            nc.scalar.copy(S128[:], S_ps[:])
            rinv128 = wpool.tile([128, 1], f32, tag="rinv128")
            nc.vector.reciprocal(rinv128[:], S128[:])
            w_col = wpool.tile([128, 4], f32, tag="w_col")
            nc.vector.tensor_scalar_mul(w_col[:], exp4[:], rinv128[:])
            w_col_bf = wpool.tile([128, 4], bf16, tag="w_col_bf")
            nc.vector.tensor_copy(w_col_bf[:], w_col[:])

            # ---- full weighted context wctx = w @ enc  ([128,8] col) ----
            wctx_ps = sp_pool.tile([128, 8], f32, tag="sp")
            for m in range(8):
                for lc in range(4):
                    nc.tensor.matmul(wctx_ps[:, m:m + 1],
                                     enc_sb[:, lc, m * 128:(m + 1) * 128],
                                     w_col_bf[:, lc:lc + 1],
                                     start=(lc == 0), stop=(lc == 3))
            wctx_col = wpool.tile([128, 8], bf16, tag="wctx_col")
            nc.scalar.copy(wctx_col[:], wctx_ps[:])

            # ---- out = relu(Wc_sh @ [embed; wctx] + bc_sh)  ([128,1]) ---
            o_ps = sp_pool.tile([128, 1], f32, tag="sp")
            for c in range(16):
                rhs = emb_bf[:, c:c + 1] if c < 8 else \
                    wctx_col[:, c - 8:c - 7]
                nc.tensor.matmul(o_ps[:], wct_sb[:, c, :], rhs,
                                 start=(c == 0), stop=(c == 15))
            relu_sb = wpool.tile([128, 1], bf16, tag="relu_sb")
            nc.scalar.activation(relu_sb[:], o_ps[:], AF.Relu,
                                 bias=vecs_sb[:, 66:67])

            # ---- GRU gate partials (contraction over local H slice) -----
            for m in range(24):
                nc.tensor.matmul(ghh_ps[:, 24 + m:25 + m],
                                 wiht_sb[:, m * 128:(m + 1) * 128],
                                 relu_sb[:], start=True, stop=True)
            gigh_sb = wpool.tile([128, 48], f32, tag="gigh_sb")
            nc.vector.tensor_copy(gigh_sb[:], ghh_ps[:])

            # ---- AllReduce: [gi | gh] partials --------------------------
            ccg_in = dpool.tile([128, 48], f32)
            ccg_out = dpool.tile([128, 48], f32)
            nc.sync.dma_start(ccg_in[:], gigh_sb[:])
            nc.gpsimd.collective_compute(
                "AllReduce", mybir.AluOpType.add, replica_groups=RG,
                ins=[ccg_in.opt()], outs=[ccg_out.opt()])
            gigh = wpool.tile([128, 48], f32, tag="gigh")
            nc.gpsimd.dma_start(gigh[:], ccg_out.opt())

            # ---- gates: r,z = sig(gi+gh+b); n = tanh(gi_n+b + r*(gh_n+b))
            trz = wpool.tile([128, 16], f32, tag="trz")
            nc.vector.tensor_add(trz[:], gigh[:, 24:40], gigh[:, 0:16])
            trz2 = wpool.tile([128, 16], f32, tag="trz2")
            nc.vector.tensor_add(trz2[:], trz[:], vecs_sb[:, 33:49])
            rz = wpool.tile([128, 16], f32, tag="rz")
            nc.scalar.activation(rz[:], trz2[:], AF.Sigmoid)
            ghn = wpool.tile([128, 8], f32, tag="ghn")
            nc.vector.tensor_add(ghn[:], gigh[:, 16:24], vecs_sb[:, 57:65])
            tn = wpool.tile([128, 8], f32, tag="tn")
            nc.vector.tensor_mul(tn[:], rz[:, 0:8], ghn[:])
            tn2 = wpool.tile([128, 8], f32, tag="tn2")
            nc.vector.tensor_add(tn2[:], tn[:], gigh[:, 40:48])
            tn3 = wpool.tile([128, 8], f32, tag="tn3")
            nc.vector.tensor_add(tn3[:], tn2[:], vecs_sb[:, 49:57])
            nn_t = wpool.tile([128, 8], f32, tag="nn_t")
            nc.scalar.activation(nn_t[:], tn3[:], AF.Tanh)
            dd = wpool.tile([128, 8], f32, tag="dd")
            nc.vector.tensor_sub(dd[:], vecs_sb[:, 24:32], nn_t[:])
            ee = wpool.tile([128, 8], f32, tag="ee")
            nc.vector.tensor_mul(ee[:], rz[:, 8:16], dd[:])
            hnew = wpool.tile([128, 8], f32, tag="hnew")
            nc.vector.tensor_add(hnew[:], nn_t[:], ee[:])
            nc.gpsimd.dma_start(hnew_o.ap(), hnew[:])
            hnew_bf = wpool.tile([128, 8], bf16, tag="hnew_bf")
            nc.vector.tensor_copy(hnew_bf[:], hnew[:])

            # ---- logits: 4-way column-tiled matmul ----------------------
            # stationary h is replicated across each group's 32 columns, so
            # every PSUM partition carries a copy of its group's logits row
            # and the epilogue runs as dense 128-partition ops.
            hrep = cpool.tile([128, 8, 32], bf16)
            for k in range(8):
                nc.vector.tensor_scalar_mul(hrep[:, k, :], ones32_bf[:],
                                            hnew[:, k:k + 1])
            logits4 = cpool.tile([128, G_COLS], f32)
            se4 = wpool.tile([128, 1], f32, tag="se4")
            for cc in range(4):
                lg_ps = lg_pool.tile([128, CH], f32, tag="lg")
                for k in range(8):
                    for g in range(4):
                        c = 4 * cc + g
                        if c >= N_CH:
                            continue
                        csz = min(CH, V_SH - c * CH)
                        nc.tensor.matmul(lg_ps[32 * g:32 * g + 32, 0:csz],
                                         hrep[:, k, :],
                                         wot_sb[:, k,
                                                c * CH:c * CH + csz],
                                         start=(k == 0), stop=(k == 7),
                                         skip_group_check=True,
                                         tile_position=(0, 32 * g))
                if cc < 3:
                    nc.vector.tensor_add(
                        logits4[:, cc * CH:(cc + 1) * CH],
                        lg_ps[:, 0:CH], bo4_sb[:, cc * CH:(cc + 1) * CH])
                else:
                    csz = V_SH - 12 * CH
                    nc.vector.tensor_add(
                        logits4[0:32, 3 * CH:3 * CH + csz],
                        lg_ps[0:32, 0:csz],
                        bo4_sb[0:32, 3 * CH:3 * CH + csz])

            # ---- dense exp + fused row-sums, then global AllGather ------
            etmp = cpool.tile([128, G_COLS], f32)
            se4b = wpool.tile([128, 1], f32, tag="se4b")
            nc.scalar.activation(etmp[:, 0:3 * CH], logits4[:, 0:3 * CH],
                                 AF.Exp, accum_out=se4[:])
            nc.scalar.activation(etmp[0:32, 3 * CH:G_COLS],
                                 logits4[0:32, 3 * CH:G_COLS],
                                 AF.Exp, accum_out=se4b[0:32, :])
            nc.vector.tensor_add(se4[0:1, :], se4[0:1, :], se4b[0:1, :])
            ccs_in = dpool.tile([1, 8], f32)
            ccs_out = dpool.tile([8, 8], f32)
            nc.gpsimd.dma_start(ccs_in[:], zz[:])
            nc.gpsimd.dma_start(
                ccs_in[0:1, 0:4],
                se4[:, :].rearrange("(a b) f -> a b f", b=32)[:, 0:1, :])
            nc.gpsimd.collective_compute(
                "AllGather", mybir.AluOpType.bypass, replica_groups=RG,
                ins=[ccs_in.opt()], outs=[ccs_out.opt()])
            s2row = wpool.tile([1, 32], f32, tag="s2row")
            nc.gpsimd.dma_start(s2row[:], ccs_out.opt()[:, 0:4])
            S2 = wpool.tile([1, 1], f32, tag="S2")
            nc.vector.reduce_sum(S2[:], s2row[:], axis=mybir.AxisListType.X)
            logS = wpool.tile([1, 1], f32, tag="logS")
            nc.scalar.activation(logS[:], S2[:], AF.Ln)
            ls_ps = sp_pool.tile([128, 1], f32, tag="sp")
            nc.tensor.matmul(ls_ps[:], aux_sb[0:1, 0:128], logS[:],
                             start=True, stop=True)
            logs128 = wpool.tile([128, 1], f32, tag="logs128")
            nc.scalar.copy(logs128[:], ls_ps[:])
            nlogs32 = wpool.tile([128, 1], f32, tag="nlogs32")
            nc.scalar.activation(nlogs32[0:32, :], logs128[0:32, :],
                                 AF.Copy, scale=-1.0)

            # out = logits - ln(S): two dense subtracts
            nc.vector.tensor_scalar_sub(logits4[:, 0:3 * CH],
                                        logits4[:, 0:3 * CH], logs128[:])
            nc.scalar.activation(logits4[0:32, 3 * CH:G_COLS],
                                 logits4[0:32, 3 * CH:G_COLS],
                                 AF.Identity, bias=nlogs32[0:32, :])

            # ---- output DMAs -------------------------------------------
            dst_all = out_sl.ap()[:, 0:4 * 3 * CH].rearrange(
                "p (cc g f) -> p g cc f", cc=3, g=4)
            src_all = logits4[:, 0:3 * CH].rearrange(
                "(a b) (cc f) -> a b cc f", b=32, f=CH)[:, 0:1, :, :]
            nc.sync.dma_start(dst_all, src_all)
            nc.sync.dma_start(out_sl.ap()[:, 12 * CH:V_SH],
                              logits4[0:1, 3 * CH:G_COLS])

            # ---- attention weights output (off the critical path) -------
            # w_o element d=128t+64e+j <- w_col[64e+j, t]
            wo_v = w_o.ap().rearrange("p (t e j) -> p e j t", e=2, j=64)
            nc.gpsimd.dma_start(wo_v[:, 0:1], w_col[0:64, 0:4])
            nc.gpsimd.dma_start(wo_v[:, 1:2], w_col[64:128, 0:4])

    nc.compile()
    return nc


def _col(v, ncols):
    return np.ascontiguousarray(v.reshape(ncols, 128).T)


def _prep_in_maps(inputs):
    f32 = np.float32
    x = np.asarray(inputs["x"]).reshape(-1)
    hidden = np.asarray(inputs["hidden"], f32).reshape(H)
    enc_full = np.ascontiguousarray(np.asarray(inputs["encoder_outputs"], f32))
    emb = np.asarray(inputs["emb"], f32)
    Ww = np.asarray(inputs["Ww"], f32)
    bw = np.asarray(inputs["bw"], f32)
    Wc = np.asarray(inputs["Wc"], f32)
    bc = np.asarray(inputs["bc"], f32)
    Wih = np.asarray(inputs["Wih"], f32)
    Whh = np.asarray(inputs["Whh"], f32)
    bih = np.asarray(inputs["bih"], f32)
    bhh = np.asarray(inputs["bhh"], f32)
    Wo = np.asarray(inputs["Wo"], f32)
    bo = np.asarray(inputs["bo"], f32)

    embed = emb[int(x[0])]
    cat1 = np.concatenate([embed, hidden])

    vecs = np.zeros((128, 68), f32)
    vecs[:, 0:16] = _col(cat1, 16)
    vecs[:, 16:24] = _col(embed, 8)
    vecs[:, 24:32] = _col(hidden, 8)
    vecs[:, 32] = 1.0
    vecs[:, 33:49] = _col((bih + bhh)[0:2048], 16)
    vecs[:, 49:57] = _col(bih[2048:], 8)
    vecs[:, 57:65] = _col(bhh[2048:], 8)

    aux = np.ones((1, 128), f32)

    pad = V_PAD - V
    Wo_pad = np.concatenate([Wo, np.zeros((pad, H), f32)], axis=0)
    bo_pad = np.concatenate([bo, np.full((pad,), -1e4, f32)])

    in_maps = []
    for r in range(NCORES):
        vr = vecs.copy()
        vr[0:64, 65] = bw[r * 64:(r + 1) * 64]
        vr[:, 66] = bc[r * 128:(r + 1) * 128]
        hs = slice(r * 128, (r + 1) * 128)
        bo_sh = bo_pad[r * V_SH:(r + 1) * V_SH]
        bo4m = np.zeros((4, G_COLS), f32)
        for c in range(N_CH):
            g, cc = c % 4, c // 4
            csz = min(CH, V_SH - c * CH)
            bo4m[g, cc * CH:cc * CH + csz] = bo_sh[c * CH:c * CH + csz]
        bo4m = np.repeat(bo4m, 32, axis=0)
        in_maps.append({
            "vecs": vr,
            "aux_row": aux,
            "wwt": np.ascontiguousarray(Ww[r * 64:(r + 1) * 64, :].T),
            "enc": enc_full.astype(_BF16),
            "wct": np.ascontiguousarray(Wc[hs, :].T).astype(_BF16),
            "wiht": np.ascontiguousarray(Wih[:, hs].T).astype(_BF16),
            "whht": np.ascontiguousarray(Whh[:, hs].T).astype(_BF16),
            "hloc": hidden[hs].reshape(128, 1).astype(_BF16),
            "wot": np.ascontiguousarray(
                Wo_pad[r * V_SH:(r + 1) * V_SH, :].T).astype(_BF16),
            "bo4": bo4m,
        })
    return in_maps


def _get_nc():
    if "nc" not in _cache:
        _cache["nc"] = _build()
    return _cache["nc"]


def _assemble(results):
    out = np.concatenate(
        [results[r]["out_sl"].reshape(-1) for r in range(NCORES)])[:V]
    out = np.ascontiguousarray(out.reshape(1, V), dtype=np.float32)
    h_new = np.ascontiguousarray(
        results[0]["hnew_o"].T.reshape(1, 1, H), dtype=np.float32)
    weights = np.ascontiguousarray(
        results[0]["w_o"].reshape(1, L), dtype=np.float32)
    return out, h_new, weights


def kernel(**inputs):
    from concourse.bass_utils import run_bass_kernel_spmd

    nc = _get_nc()
    in_maps = _prep_in_maps(inputs)
    res = run_bass_kernel_spmd(nc, in_maps, list(range(NCORES)))
    return _assemble(res.results)


# revision 23
# speedup vs baseline: 1.0553x; 1.0553x over previous
"""Trainium2 Bass kernel for nn_DecoderGRUWeighted (batch-1 GRU decoder step).

Strategy (8 NeuronCores, SPMD):
  - Vocab dim of the output projection Wo (50257x1024, the dominant memory
    traffic) is sharded 8 ways; each core computes a [1, V/8] logits slice
    with a 4-way column-tiled PE matmul (4 concurrent streams).
  - Attention scores are sharded by L and combined with one tiny AllGather;
    each core then computes the full softmax + context locally (encoder
    replicated). GRU gate partials are combined with one AllReduce.
  - A dependency-free dummy AllGather fires at t=0 so the cross-core
    entry barrier + first-collective setup overlap the weight streaming.
  - All length-D vectors live on chip in "column layout": SBUF tile
    [128, D/128] with element d at (partition d%128, column d//128), so every
    matvec uses natural [128,128] weight tiles as the PE stationary operand
    and [128,1] vector columns as the moving operand, with no transposes.
  - log_softmax is computed without max subtraction (logits are O(1) here):
    out = logits - ln(sum_exp), with the global sum reduced via AllGather.
  - Wo^T / Wih^T / Whh^T / Wc^T / encoder are stored fp16 (all values are
    O(1), so fp16's 10-bit mantissa beats bf16 at the same speed) to halve
    HBM traffic and double PE throughput vs f32; Ww and all the
    softmax/log-softmax math stay f32.
"""

import sys

if "/opt/trn_rl_repo" not in sys.path:
    sys.path.insert(0, "/opt/trn_rl_repo")

import numpy as np

H = 1024
V = 50257
L = 512
NCORES = 8
V_SH = 6283          # ceil(V / 8); global pad = 50264 (7 zero rows on core 7)
V_PAD = V_SH * NCORES
CH = 512             # logits chunk (one PSUM bank of f32)
N_CH = 13            # 12*512 + 139
G_COLS = 1675        # per-group row length: 3*512 + 139 (group 0 holds the tail)

_F16 = np.float16

_cache: dict = {}


def _build():
    import concourse.bacc as bacc
    import concourse.tile as tile
    from concourse import mybir

    f32 = mybir.dt.float32
    bf16 = mybir.dt.float16
    AF = mybir.ActivationFunctionType

    nc = bacc.Bacc("TRN2", target_bir_lowering=False, debug=False,
                   num_devices=NCORES)

    # ---- I/O ------------------------------------------------------------
    # vecs columns: 0:16 cat1 | 16:24 embed | 24:32 hidden | 32 ones |
    #               33:49 bih+bhh (r,z) | 49:57 bih_n | 57:65 bhh_n |
    #               65 bw shard (partitions 0:64) | 66 bc shard
    vecs = nc.dram_tensor("vecs", [128, 68], f32, kind="ExternalInput")
    aux_row = nc.dram_tensor("aux_row", [1, 128], f32, kind="ExternalInput")
    wwt = nc.dram_tensor("wwt", [2048, 64], f32, kind="ExternalInput")
    enc = nc.dram_tensor("enc", [512, 1024], bf16, kind="ExternalInput")
    wct = nc.dram_tensor("wct", [2048, 128], bf16, kind="ExternalInput")
    wiht = nc.dram_tensor("wiht", [128, 3072], bf16, kind="ExternalInput")
    whht = nc.dram_tensor("whht", [128, 3072], bf16, kind="ExternalInput")
    hloc = nc.dram_tensor("hloc", [128, 1], bf16, kind="ExternalInput")
    wot = nc.dram_tensor("wot", [1024, V_SH], bf16, kind="ExternalInput")
    bo4 = nc.dram_tensor("bo4", [128, G_COLS], f32, kind="ExternalInput")

    out_sl = nc.dram_tensor("out_sl", [1, V_SH], f32, kind="ExternalOutput")
    hnew_o = nc.dram_tensor("hnew_o", [128, 8], f32, kind="ExternalOutput")
    w_o = nc.dram_tensor("w_o", [1, 512], f32, kind="ExternalOutput")

    RG = [list(range(NCORES))]

    with tile.TileContext(nc) as tc:
        with (
            tc.tile_pool(name="const", bufs=1) as cpool,
            tc.tile_pool(name="work", bufs=2) as wpool,
            tc.tile_pool(name="lgp", bufs=4, space="PSUM") as lg_pool,
            tc.tile_pool(name="spp", bufs=2, space="PSUM") as sp_pool,
            tc.tile_pool(name="dram", bufs=1, space="DRAM") as dpool,
        ):
            zz = cpool.tile([1, 8], f32)
            nc.vector.memset(zz[:], 0.0)
            ones32_bf = cpool.tile([128, 32], bf16)
            nc.vector.memset(ones32_bf[:], 1.0)
            ones128 = cpool.tile([128, 128], f32)
            nc.vector.memset(ones128[:], 1.0)

            # ---- stage the small inputs (chain-critical DMAs first) -----
            vecs_sb = cpool.tile([128, 68], f32)
            nc.sync.dma_start(vecs_sb[:], vecs.ap())
            aux_sb = cpool.tile([1, 128], f32)
            nc.sync.dma_start(aux_sb[:], aux_row.ap())
            wwt_sb = cpool.tile([128, 16, 64], f32)
            nc.sync.dma_start(
                wwt_sb[:], wwt.ap().rearrange("(c p) f -> p c f", p=128))
            hloc_sb = cpool.tile([128, 1], bf16)
            nc.sync.dma_start(hloc_sb[:], hloc.ap())
            ones_col = vecs_sb[:, 32:33]

            # bf16 copy of the embedding columns (feeds bf16 Wc matmul)
            emb_bf = cpool.tile([128, 8], bf16)
            nc.vector.tensor_copy(emb_bf[:], vecs_sb[:, 16:24])

            # ---- attention scores s = Ww_sh @ cat1 + bw_sh  ([64,1]) ----
            s_ps = sp_pool.tile([64, 1], f32, tag="sp")
            for c in range(16):
                nc.tensor.matmul(s_ps[:], wwt_sb[:, c, :],
                                 vecs_sb[:, c:c + 1],
                                 start=(c == 0), stop=(c == 15))
            s_sb = wpool.tile([64, 1], f32, tag="s_sb")
            nc.scalar.activation(s_sb[:], s_ps[:], AF.Identity,
                                 bias=vecs_sb[0:64, 65:66])

            # ---- AllGather #1: local scores -> all 512 scores -----------
            cc1_in = dpool.tile([1, 64], f32)
            cc1_out = dpool.tile([8, 64], f32)
            nc.sync.dma_start(cc1_in[:], s_sb[:])
            nc.gpsimd.collective_compute(
                "AllGather", mybir.AluOpType.bypass, replica_groups=RG,
                ins=[cc1_in.opt()], outs=[cc1_out.opt()])

            enc_sb = cpool.tile([128, 4, 1024], bf16)
            nc.sync.dma_start(
                enc_sb[:], enc.ap().rearrange("(c p) f -> p c f", p=128))
            wct_sb = cpool.tile([128, 16, 128], bf16)
            nc.sync.dma_start(
                wct_sb[:], wct.ap().rearrange("(c p) f -> p c f", p=128))
            wiht_sb = cpool.tile([128, 3072], bf16)
            nc.sync.dma_start(wiht_sb[:], wiht.ap())
            whht_sb = cpool.tile([128, 3072], bf16)
            nc.sync.dma_start(whht_sb[:], whht.ap())
            bo4_sb = cpool.tile([128, G_COLS], f32)
            nc.sync.dma_start(bo4_sb[:], bo4.ap())

            # gh partials depend only on inputs: run during the AllGather
            # wait (useful work + keeps the PE array warm)
            ghh_ps = sp_pool.tile([128, 48], f32, tag="sp")
            for m in range(24):
                nc.tensor.matmul(ghh_ps[:, m:m + 1],
                                 whht_sb[:, m * 128:(m + 1) * 128],
                                 hloc_sb[:], start=True, stop=True)

            # ---- big Wo^T shard: 8 contraction tiles, streamed early ----
            wot_sb = cpool.tile([128, 8, V_SH], bf16)
            for k in range(8):
                nc.sync.dma_start(
                    wot_sb[:, k, :], wot.ap()[k * 128:(k + 1) * 128, :])

            # readback into column layout [128, 4]: element l=128t+64e+j at
            # (partition 64e+j, col t); gathered row r=2t+e holds j=0..63
            scores_col = wpool.tile([128, 4], f32, tag="scores_col")
            cc1_v = cc1_out.opt().rearrange("(t e) j -> e j t", e=2)
            nc.gpsimd.dma_start(scores_col[0:64, 0:4], cc1_v[0:1])
            nc.gpsimd.dma_start(scores_col[64:128, 0:4], cc1_v[1:2])

            # full softmax (local): w = exp(s) / sum(exp(s))
            exp4 = wpool.tile([128, 4], f32, tag="exp4")
            acc4 = wpool.tile([128, 1], f32, tag="acc4")
            nc.scalar.activation(exp4[:], scores_col[:], AF.Exp,
                                 accum_out=acc4[:])
            S_ps = sp_pool.tile([128, 1], f32, tag="sp")
            nc.tensor.matmul(S_ps[:], ones128[:], acc4[:],
                             start=True, stop=True)
            S128 = wpool.tile([128, 1], f32, tag="S128")
            nc.scalar.copy(S128[:], S_ps[:])
            rinv128 = wpool.tile([128, 1], f32, tag="rinv128")
            nc.vector.reciprocal(rinv128[:], S128[:])
            w_col = wpool.tile([128, 4], f32, tag="w_col")
            nc.vector.tensor_scalar_mul(w_col[:], exp4[:], rinv128[:])
            w_col_bf = wpool.tile([128, 4], bf16, tag="w_col_bf")
            nc.vector.tensor_copy(w_col_bf[:], w_col[:])

            # ---- full weighted context wctx = w @ enc  ([128,8] col) ----
            wctx_ps = sp_pool.tile([128, 8], f32, tag="sp")
            for m in range(8):
                for lc in range(4):
                    nc.tensor.matmul(wctx_ps[:, m:m + 1],
                                     enc_sb[:, lc, m * 128:(m + 1) * 128],
                                     w_col_bf[:, lc:lc + 1],
                                     start=(lc == 0), stop=(lc == 3))
            wctx_col = wpool.tile([128, 8], bf16, tag="wctx_col")
            nc.scalar.copy(wctx_col[:], wctx_ps[:])

            # ---- out = relu(Wc_sh @ [embed; wctx] + bc_sh)  ([128,1]) ---
            o_ps = sp_pool.tile([128, 1], f32, tag="sp")
            for c in range(16):
                rhs = emb_bf[:, c:c + 1] if c < 8 else \
                    wctx_col[:, c - 8:c - 7]
                nc.tensor.matmul(o_ps[:], wct_sb[:, c, :], rhs,
                                 start=(c == 0), stop=(c == 15))
            relu_sb = wpool.tile([128, 1], bf16, tag="relu_sb")
            nc.scalar.activation(relu_sb[:], o_ps[:], AF.Relu,
                                 bias=vecs_sb[:, 66:67])

            # ---- GRU gate partials (contraction over local H slice) -----
            for m in range(24):
                nc.tensor.matmul(ghh_ps[:, 24 + m:25 + m],
                                 wiht_sb[:, m * 128:(m + 1) * 128],
                                 relu_sb[:], start=True, stop=True)
            gigh_sb = wpool.tile([128, 48], f32, tag="gigh_sb")
            nc.vector.tensor_copy(gigh_sb[:], ghh_ps[:])

            # ---- AllReduce: [gi | gh] partials --------------------------
            ccg_in = dpool.tile([128, 48], f32)
            ccg_out = dpool.tile([128, 48], f32)
            nc.sync.dma_start(ccg_in[:], gigh_sb[:])
            nc.gpsimd.collective_compute(
                "AllReduce", mybir.AluOpType.add, replica_groups=RG,
                ins=[ccg_in.opt()], outs=[ccg_out.opt()])
            gigh = wpool.tile([128, 48], f32, tag="gigh")
            nc.gpsimd.dma_start(gigh[:], ccg_out.opt())

            # ---- gates: r,z = sig(gi+gh+b); n = tanh(gi_n+b + r*(gh_n+b))
            trz = wpool.tile([128, 16], f32, tag="trz")
            nc.vector.tensor_add(trz[:], gigh[:, 24:40], gigh[:, 0:16])
            trz2 = wpool.tile([128, 16], f32, tag="trz2")
            nc.vector.tensor_add(trz2[:], trz[:], vecs_sb[:, 33:49])
            rz = wpool.tile([128, 16], f32, tag="rz")
            nc.scalar.activation(rz[:], trz2[:], AF.Sigmoid)
            ghn = wpool.tile([128, 8], f32, tag="ghn")
            nc.vector.tensor_add(ghn[:], gigh[:, 16:24], vecs_sb[:, 57:65])
            tn = wpool.tile([128, 8], f32, tag="tn")
            nc.vector.tensor_mul(tn[:], rz[:, 0:8], ghn[:])
            tn2 = wpool.tile([128, 8], f32, tag="tn2")
            nc.vector.tensor_add(tn2[:], tn[:], gigh[:, 40:48])
            tn3 = wpool.tile([128, 8], f32, tag="tn3")
            nc.vector.tensor_add(tn3[:], tn2[:], vecs_sb[:, 49:57])
            nn_t = wpool.tile([128, 8], f32, tag="nn_t")
            nc.scalar.activation(nn_t[:], tn3[:], AF.Tanh)
            dd = wpool.tile([128, 8], f32, tag="dd")
            nc.vector.tensor_sub(dd[:], vecs_sb[:, 24:32], nn_t[:])
            ee = wpool.tile([128, 8], f32, tag="ee")
            nc.vector.tensor_mul(ee[:], rz[:, 8:16], dd[:])
            hnew = wpool.tile([128, 8], f32, tag="hnew")
            nc.vector.tensor_add(hnew[:], nn_t[:], ee[:])
            nc.gpsimd.dma_start(hnew_o.ap(), hnew[:])

            # ---- logits: 4-way column-tiled matmul ----------------------
            # stationary h is replicated across each group's 32 columns, so
            # every PSUM partition carries a copy of its group's logits row
            # and the epilogue runs as dense 128-partition ops.
            hrep = cpool.tile([128, 8, 32], bf16)
            for k in range(8):
                nc.vector.tensor_scalar_mul(hrep[:, k, :], ones32_bf[:],
                                            hnew[:, k:k + 1])
            logits4 = cpool.tile([128, G_COLS], f32)
            se4 = wpool.tile([128, 1], f32, tag="se4")
            for cc in range(4):
                lg_ps = lg_pool.tile([128, CH], f32, tag="lg")
                for k in range(8):
                    for g in range(4):
                        c = 4 * cc + g
                        if c >= N_CH:
                            continue
                        csz = min(CH, V_SH - c * CH)
                        nc.tensor.matmul(lg_ps[32 * g:32 * g + 32, 0:csz],
                                         hrep[:, k, :],
                                         wot_sb[:, k,
                                                c * CH:c * CH + csz],
                                         start=(k == 0), stop=(k == 7),
                                         skip_group_check=True,
                                         tile_position=(0, 32 * g))
                if cc < 3:
                    nc.vector.tensor_add(
                        logits4[:, cc * CH:(cc + 1) * CH],
                        lg_ps[:, 0:CH], bo4_sb[:, cc * CH:(cc + 1) * CH])
                else:
                    csz = V_SH - 12 * CH
                    nc.vector.tensor_add(
                        logits4[0:32, 3 * CH:3 * CH + csz],
                        lg_ps[0:32, 0:csz],
                        bo4_sb[0:32, 3 * CH:3 * CH + csz])

            # ---- dense exp + fused row-sums, then global AllGather ------
            etmp = cpool.tile([128, G_COLS], f32)
            se4b = wpool.tile([128, 1], f32, tag="se4b")
            nc.scalar.activation(etmp[:, 0:3 * CH], logits4[:, 0:3 * CH],
                                 AF.Exp, accum_out=se4[:])
            nc.scalar.activation(etmp[0:32, 3 * CH:G_COLS],
                                 logits4[0:32, 3 * CH:G_COLS],
                                 AF.Exp, accum_out=se4b[0:32, :])
            nc.vector.tensor_add(se4[0:1, :], se4[0:1, :], se4b[0:1, :])
            ccs_in = dpool.tile([1, 8], f32)
            ccs_out = dpool.tile([8, 8], f32)
            nc.gpsimd.dma_start(ccs_in[:], zz[:])
            nc.gpsimd.dma_start(
                ccs_in[0:1, 0:4],
                se4[:, :].rearrange("(a b) f -> a b f", b=32)[:, 0:1, :])
            nc.gpsimd.collective_compute(
                "AllGather", mybir.AluOpType.bypass, replica_groups=RG,
                ins=[ccs_in.opt()], outs=[ccs_out.opt()])
            s2row = wpool.tile([1, 32], f32, tag="s2row")
            nc.gpsimd.dma_start(s2row[:], ccs_out.opt()[:, 0:4])
            S2 = wpool.tile([1, 1], f32, tag="S2")
            nc.vector.reduce_sum(S2[:], s2row[:], axis=mybir.AxisListType.X)
            logS = wpool.tile([1, 1], f32, tag="logS")
            nc.scalar.activation(logS[:], S2[:], AF.Ln)
            ls_ps = sp_pool.tile([128, 1], f32, tag="sp")
            nc.tensor.matmul(ls_ps[:], aux_sb[0:1, 0:128], logS[:],
                             start=True, stop=True)
            logs128 = wpool.tile([128, 1], f32, tag="logs128")
            nc.scalar.copy(logs128[:], ls_ps[:])
            nlogs32 = wpool.tile([128, 1], f32, tag="nlogs32")
            nc.scalar.activation(nlogs32[0:32, :], logs128[0:32, :],
                                 AF.Copy, scale=-1.0)

            # out = logits - ln(S): two dense subtracts
            nc.vector.tensor_scalar_sub(logits4[:, 0:3 * CH],
                                        logits4[:, 0:3 * CH], logs128[:])
            nc.scalar.activation(logits4[0:32, 3 * CH:G_COLS],
                                 logits4[0:32, 3 * CH:G_COLS],
                                 AF.Identity, bias=nlogs32[0:32, :])

            # ---- output DMAs -------------------------------------------
            dst_all = out_sl.ap()[:, 0:4 * 3 * CH].rearrange(
                "p (cc g f) -> p g cc f", cc=3, g=4)
            src_all = logits4[:, 0:3 * CH].rearrange(
                "(a b) (cc f) -> a b cc f", b=32, f=CH)[:, 0:1, :, :]
            nc.sync.dma_start(dst_all, src_all)
            nc.sync.dma_start(out_sl.ap()[:, 12 * CH:V_SH],
                              logits4[0:1, 3 * CH:G_COLS])

            # ---- attention weights output (off the critical path) -------
            # w_o element d=128t+64e+j <- w_col[64e+j, t]
            wo_v = w_o.ap().rearrange("p (t e j) -> p e j t", e=2, j=64)
            nc.gpsimd.dma_start(wo_v[:, 0:1], w_col[0:64, 0:4])
            nc.gpsimd.dma_start(wo_v[:, 1:2], w_col[64:128, 0:4])

    nc.compile()
    return nc


def _col(v, ncols):
    return np.ascontiguousarray(v.reshape(ncols, 128).T)


def _prep_in_maps(inputs):
    f32 = np.float32
    x = np.asarray(inputs["x"]).reshape(-1)
    hidden = np.asarray(inputs["hidden"], f32).reshape(H)
    enc_full = np.ascontiguousarray(np.asarray(inputs["encoder_outputs"], f32))
    emb = np.asarray(inputs["emb"], f32)
    Ww = np.asarray(inputs["Ww"], f32)
    bw = np.asarray(inputs["bw"], f32)
    Wc = np.asarray(inputs["Wc"], f32)
    bc = np.asarray(inputs["bc"], f32)
    Wih = np.asarray(inputs["Wih"], f32)
    Whh = np.asarray(inputs["Whh"], f32)
    bih = np.asarray(inputs["bih"], f32)
    bhh = np.asarray(inputs["bhh"], f32)
    Wo = np.asarray(inputs["Wo"], f32)
    bo = np.asarray(inputs["bo"], f32)

    embed = emb[int(x[0])]
    cat1 = np.concatenate([embed, hidden])

    vecs = np.zeros((128, 68), f32)
    vecs[:, 0:16] = _col(cat1, 16)
    vecs[:, 16:24] = _col(embed, 8)
    vecs[:, 24:32] = _col(hidden, 8)
    vecs[:, 32] = 1.0
    vecs[:, 33:49] = _col((bih + bhh)[0:2048], 16)
    vecs[:, 49:57] = _col(bih[2048:], 8)
    vecs[:, 57:65] = _col(bhh[2048:], 8)

    aux = np.ones((1, 128), f32)

    pad = V_PAD - V
    Wo_pad = np.concatenate([Wo, np.zeros((pad, H), f32)], axis=0)
    bo_pad = np.concatenate([bo, np.full((pad,), -1e4, f32)])

    in_maps = []
    for r in range(NCORES):
        vr = vecs.copy()
        vr[0:64, 65] = bw[r * 64:(r + 1) * 64]
        vr[:, 66] = bc[r * 128:(r + 1) * 128]
        hs = slice(r * 128, (r + 1) * 128)
        bo_sh = bo_pad[r * V_SH:(r + 1) * V_SH]
        bo4m = np.zeros((4, G_COLS), f32)
        for c in range(N_CH):
            g, cc = c % 4, c // 4
            csz = min(CH, V_SH - c * CH)
            bo4m[g, cc * CH:cc * CH + csz] = bo_sh[c * CH:c * CH + csz]
        bo4m = np.repeat(bo4m, 32, axis=0)
        in_maps.append({
            "vecs": vr,
            "aux_row": aux,
            "wwt": np.ascontiguousarray(Ww[r * 64:(r + 1) * 64, :].T),
            "enc": enc_full.astype(_F16),
            "wct": np.ascontiguousarray(Wc[hs, :].T).astype(_F16),
            "wiht": np.ascontiguousarray(Wih[:, hs].T).astype(_F16),
            "whht": np.ascontiguousarray(Whh[:, hs].T).astype(_F16),
            "hloc": hidden[hs].reshape(128, 1).astype(_F16),
            "wot": np.ascontiguousarray(
                Wo_pad[r * V_SH:(r + 1) * V_SH, :].T).astype(_F16),
            "bo4": bo4m,
        })
    return in_maps


def _get_nc():
    if "nc" not in _cache:
        _cache["nc"] = _build()
    return _cache["nc"]


def _assemble(results):
    out = np.concatenate(
        [results[r]["out_sl"].reshape(-1) for r in range(NCORES)])[:V]
    out = np.ascontiguousarray(out.reshape(1, V), dtype=np.float32)
    h_new = np.ascontiguousarray(
        results[0]["hnew_o"].T.reshape(1, 1, H), dtype=np.float32)
    weights = np.ascontiguousarray(
        results[0]["w_o"].reshape(1, L), dtype=np.float32)
    return out, h_new, weights


def kernel(**inputs):
    from concourse.bass_utils import run_bass_kernel_spmd

    nc = _get_nc()
    in_maps = _prep_in_maps(inputs)
    res = run_bass_kernel_spmd(nc, in_maps, list(range(NCORES)))
    return _assemble(res.results)


# revision 25
# speedup vs baseline: 1.0965x; 1.0390x over previous
"""Trainium2 Bass kernel for nn_DecoderGRUWeighted (batch-1 GRU decoder step).

Strategy (8 NeuronCores, SPMD):
  - Vocab dim of the output projection Wo (50257x1024, the dominant memory
    traffic) is sharded 8 ways; each core computes a [1, V/8] logits slice
    with a 4-way column-tiled PE matmul (4 concurrent streams).
  - Attention scores are sharded by L and combined with one tiny AllGather;
    each core then computes the full softmax + context locally (encoder
    replicated). GRU gate partials are combined with one AllReduce.
  - A dependency-free dummy AllGather fires at t=0 so the cross-core
    entry barrier + first-collective setup overlap the weight streaming.
  - All length-D vectors live on chip in "column layout": SBUF tile
    [128, D/128] with element d at (partition d%128, column d//128), so every
    matvec uses natural [128,128] weight tiles as the PE stationary operand
    and [128,1] vector columns as the moving operand, with no transposes.
  - log_softmax is computed without max subtraction (logits are O(1) here):
    out = logits - ln(sum_exp), with the global sum reduced via AllGather.
  - Wo^T / Wih^T / Whh^T / Wc^T / encoder are stored fp16 (all values are
    O(1), so fp16's 10-bit mantissa beats bf16 at the same speed) to halve
    HBM traffic and double PE throughput vs f32; Ww and all the
    softmax/log-softmax math stay f32.
"""

import sys

if "/opt/trn_rl_repo" not in sys.path:
    sys.path.insert(0, "/opt/trn_rl_repo")

import numpy as np

H = 1024
V = 50257
L = 512
NCORES = 8
V_SH = 6283          # ceil(V / 8); global pad = 50264 (7 zero rows on core 7)
V_PAD = V_SH * NCORES
CH = 512             # logits chunk (one PSUM bank of f32)
N_CH = 13            # 12*512 + 139
G_COLS = 1675        # per-group row length: 3*512 + 139 (group 0 holds the tail)

_F16 = np.float16

_cache: dict = {}


def _build():
    import concourse.bacc as bacc
    import concourse.tile as tile
    from concourse import mybir

    f32 = mybir.dt.float32
    bf16 = mybir.dt.float16
    AF = mybir.ActivationFunctionType

    nc = bacc.Bacc("TRN2", target_bir_lowering=False, debug=False,
                   num_devices=NCORES)

    # ---- I/O ------------------------------------------------------------
    # vecs columns: 0:16 cat1 | 16:24 embed | 24:32 hidden | 32 ones |
    #               33:49 bih+bhh (r,z) | 49:57 bih_n | 57:65 bhh_n |
    #               65 bw shard (partitions 0:64) | 66 bc shard
    vecs = nc.dram_tensor("vecs", [128, 68], f32, kind="ExternalInput")
    aux_row = nc.dram_tensor("aux_row", [1, 128], f32, kind="ExternalInput")
    wwt = nc.dram_tensor("wwt", [2048, 64], f32, kind="ExternalInput")
    enc = nc.dram_tensor("enc", [512, 1024], bf16, kind="ExternalInput")
    wct = nc.dram_tensor("wct", [2048, 128], bf16, kind="ExternalInput")
    wiht = nc.dram_tensor("wiht", [128, 3072], bf16, kind="ExternalInput")
    whht = nc.dram_tensor("whht", [128, 3072], bf16, kind="ExternalInput")
    hloc = nc.dram_tensor("hloc", [128, 1], bf16, kind="ExternalInput")
    wot = nc.dram_tensor("wot", [1024, V_SH], bf16, kind="ExternalInput")
    bo4 = nc.dram_tensor("bo4", [128, G_COLS], f32, kind="ExternalInput")

    out_sl = nc.dram_tensor("out_sl", [1, V_SH], f32, kind="ExternalOutput")
    hnew_o = nc.dram_tensor("hnew_o", [128, 8], f32, kind="ExternalOutput")
    w_o = nc.dram_tensor("w_o", [1, 512], f32, kind="ExternalOutput")

    RG = [list(range(NCORES))]

    with tile.TileContext(nc) as tc:
        with (
            tc.tile_pool(name="const", bufs=1) as cpool,
            tc.tile_pool(name="work", bufs=2) as wpool,
            tc.tile_pool(name="lgp", bufs=4, space="PSUM") as lg_pool,
            tc.tile_pool(name="spp", bufs=2, space="PSUM") as sp_pool,
            tc.tile_pool(name="dram", bufs=1, space="DRAM") as dpool,
        ):
            zz = cpool.tile([1, 8], f32)
            nc.vector.memset(zz[:], 0.0)
            ones32_bf = cpool.tile([128, 32], bf16)
            nc.vector.memset(ones32_bf[:], 1.0)
            ones128 = cpool.tile([128, 128], f32)
            nc.vector.memset(ones128[:], 1.0)

            # ---- stage the small inputs (chain-critical DMAs first) -----
            vecs_sb = cpool.tile([128, 68], f32)
            nc.sync.dma_start(vecs_sb[:], vecs.ap())
            aux_sb = cpool.tile([1, 128], f32)
            nc.sync.dma_start(aux_sb[:], aux_row.ap())
            wwt_sb = cpool.tile([128, 16, 64], f32)
            nc.sync.dma_start(
                wwt_sb[:], wwt.ap().rearrange("(c p) f -> p c f", p=128))
            hloc_sb = cpool.tile([128, 1], bf16)
            nc.sync.dma_start(hloc_sb[:], hloc.ap())
            ones_col = vecs_sb[:, 32:33]

            # bf16 copy of the embedding columns (feeds bf16 Wc matmul)
            emb_bf = cpool.tile([128, 8], bf16)
            nc.vector.tensor_copy(emb_bf[:], vecs_sb[:, 16:24])

            # ---- attention scores s = Ww_sh @ cat1 + bw_sh  ([64,1]) ----
            s_ps = sp_pool.tile([64, 1], f32, tag="sp")
            for c in range(16):
                nc.tensor.matmul(s_ps[:], wwt_sb[:, c, :],
                                 vecs_sb[:, c:c + 1],
                                 start=(c == 0), stop=(c == 15))
            s_sb = wpool.tile([64, 1], f32, tag="s_sb")
            nc.scalar.activation(s_sb[:], s_ps[:], AF.Identity,
                                 bias=vecs_sb[0:64, 65:66])

            # ---- AllGather #1: local scores -> all 512 scores -----------
            cc1_in = dpool.tile([1, 64], f32)
            cc1_out = dpool.tile([8, 64], f32)
            nc.sync.dma_start(cc1_in[:], s_sb[:])
            nc.gpsimd.collective_compute(
                "AllGather", mybir.AluOpType.bypass, replica_groups=RG,
                ins=[cc1_in.opt()], outs=[cc1_out.opt()])

            enc_sb = cpool.tile([128, 4, 1024], bf16)
            nc.sync.dma_start(
                enc_sb[:], enc.ap().rearrange("(c p) f -> p c f", p=128))
            wct_sb = cpool.tile([128, 16, 128], bf16)
            nc.sync.dma_start(
                wct_sb[:], wct.ap().rearrange("(c p) f -> p c f", p=128))
            wiht_sb = cpool.tile([128, 3072], bf16)
            nc.sync.dma_start(wiht_sb[:], wiht.ap())
            whht_sb = cpool.tile([128, 3072], bf16)
            nc.sync.dma_start(whht_sb[:], whht.ap())
            bo4_sb = cpool.tile([128, G_COLS], f32)
            nc.sync.dma_start(bo4_sb[:], bo4.ap())

            # gh partials depend only on inputs: run during the AllGather
            # wait (useful work + keeps the PE array warm)
            ghh_ps = sp_pool.tile([128, 48], f32, tag="sp")
            for m in range(24):
                nc.tensor.matmul(ghh_ps[:, m:m + 1],
                                 whht_sb[:, m * 128:(m + 1) * 128],
                                 hloc_sb[:], start=True, stop=True)

            # ---- big Wo^T shard: 8 contraction tiles, streamed early ----
            wot_sb = cpool.tile([128, 8, V_SH], bf16)
            for k in range(8):
                nc.sync.dma_start(
                    wot_sb[:, k, :], wot.ap()[k * 128:(k + 1) * 128, :])

            # readback into column layout [128, 4]: element l=128t+64e+j at
            # (partition 64e+j, col t); gathered row r=2t+e holds j=0..63
            scores_col = wpool.tile([128, 4], f32, tag="scores_col")
            cc1_v = cc1_out.opt().rearrange("(t e) j -> e j t", e=2)
            nc.gpsimd.dma_start(scores_col[0:64, 0:4], cc1_v[0:1])
            nc.gpsimd.dma_start(scores_col[64:128, 0:4], cc1_v[1:2])

            # full softmax (local): w = exp(s) / sum(exp(s))
            exp4 = wpool.tile([128, 4], f32, tag="exp4")
            acc4 = wpool.tile([128, 1], f32, tag="acc4")
            nc.scalar.activation(exp4[:], scores_col[:], AF.Exp,
                                 accum_out=acc4[:])
            S_ps = sp_pool.tile([128, 1], f32, tag="sp")
            nc.tensor.matmul(S_ps[:], ones128[:], acc4[:],
                             start=True, stop=True)
            w_col_bf = wpool.tile([128, 4], bf16, tag="w_col_bf")
            nc.vector.tensor_copy(w_col_bf[:], exp4[:])
            S128 = wpool.tile([128, 1], f32, tag="S128")
            nc.scalar.copy(S128[:], S_ps[:])
            rinv128 = wpool.tile([128, 1], f32, tag="rinv128")
            nc.vector.reciprocal(rinv128[:], S128[:])
            w_col = wpool.tile([128, 4], f32, tag="w_col")
            nc.vector.tensor_scalar_mul(w_col[:], exp4[:], rinv128[:])

            # ---- full weighted context wctx = w @ enc  ([128,8] col) ----
            wctx_ps = sp_pool.tile([128, 8], f32, tag="sp")
            for m in range(8):
                for lc in range(4):
                    nc.tensor.matmul(wctx_ps[:, m:m + 1],
                                     enc_sb[:, lc, m * 128:(m + 1) * 128],
                                     w_col_bf[:, lc:lc + 1],
                                     start=(lc == 0), stop=(lc == 3))
            wctx_col = wpool.tile([128, 8], bf16, tag="wctx_col")
            nc.scalar.activation(wctx_col[:], wctx_ps[:], AF.Copy,
                                 scale=rinv128[:])

            # ---- out = relu(Wc_sh @ [embed; wctx] + bc_sh)  ([128,1]) ---
            o_ps = sp_pool.tile([128, 1], f32, tag="sp")
            for c in range(16):
                rhs = emb_bf[:, c:c + 1] if c < 8 else \
                    wctx_col[:, c - 8:c - 7]
                nc.tensor.matmul(o_ps[:], wct_sb[:, c, :], rhs,
                                 start=(c == 0), stop=(c == 15))
            relu_sb = wpool.tile([128, 1], bf16, tag="relu_sb")
            nc.scalar.activation(relu_sb[:], o_ps[:], AF.Relu,
                                 bias=vecs_sb[:, 66:67])

            # ---- GRU gate partials (contraction over local H slice) -----
            for m in range(24):
                nc.tensor.matmul(ghh_ps[:, 24 + m:25 + m],
                                 wiht_sb[:, m * 128:(m + 1) * 128],
                                 relu_sb[:], start=True, stop=True)
            gigh_sb = wpool.tile([128, 48], f32, tag="gigh_sb")
            nc.vector.tensor_copy(gigh_sb[:], ghh_ps[:])

            # ---- AllReduce: [gi | gh] partials --------------------------
            ccg_in = dpool.tile([128, 48], f32)
            ccg_out = dpool.tile([128, 48], f32)
            nc.sync.dma_start(ccg_in[:], gigh_sb[:])
            nc.gpsimd.collective_compute(
                "AllReduce", mybir.AluOpType.add, replica_groups=RG,
                ins=[ccg_in.opt()], outs=[ccg_out.opt()])
            gigh = wpool.tile([128, 48], f32, tag="gigh")
            nc.gpsimd.dma_start(gigh[:], ccg_out.opt())

            # ---- gates: r,z = sig(gi+gh+b); n = tanh(gi_n+b + r*(gh_n+b))
            trz = wpool.tile([128, 16], f32, tag="trz")
            nc.vector.tensor_add(trz[:], gigh[:, 24:40], gigh[:, 0:16])
            trz2 = wpool.tile([128, 16], f32, tag="trz2")
            nc.vector.tensor_add(trz2[:], trz[:], vecs_sb[:, 33:49])
            rz = wpool.tile([128, 16], f32, tag="rz")
            nc.scalar.activation(rz[:], trz2[:], AF.Sigmoid)
            ghn = wpool.tile([128, 8], f32, tag="ghn")
            nc.vector.tensor_add(ghn[:], gigh[:, 16:24], vecs_sb[:, 57:65])
            tn = wpool.tile([128, 8], f32, tag="tn")
            nc.vector.tensor_mul(tn[:], rz[:, 0:8], ghn[:])
            tn2 = wpool.tile([128, 8], f32, tag="tn2")
            nc.vector.tensor_add(tn2[:], tn[:], gigh[:, 40:48])
            tn3 = wpool.tile([128, 8], f32, tag="tn3")
            nc.vector.tensor_add(tn3[:], tn2[:], vecs_sb[:, 49:57])
            nn_t = wpool.tile([128, 8], f32, tag="nn_t")
            nc.scalar.activation(nn_t[:], tn3[:], AF.Tanh)
            dd = wpool.tile([128, 8], f32, tag="dd")
            nc.vector.tensor_sub(dd[:], vecs_sb[:, 24:32], nn_t[:])
            ee = wpool.tile([128, 8], f32, tag="ee")
            nc.vector.tensor_mul(ee[:], rz[:, 8:16], dd[:])
            hnew = wpool.tile([128, 8], f32, tag="hnew")
            nc.vector.tensor_add(hnew[:], nn_t[:], ee[:])
            nc.gpsimd.dma_start(hnew_o.ap(), hnew[:])

            # ---- logits: 4-way column-tiled matmul ----------------------
            # stationary h is replicated across each group's 32 columns, so
            # every PSUM partition carries a copy of its group's logits row
            # and the epilogue runs as dense 128-partition ops.
            hrepA = cpool.tile([128, 4, 32], bf16)
            hrepB = cpool.tile([128, 4, 32], bf16)
            for k in range(8):
                hr = hrepA[:, k, :] if k < 4 else hrepB[:, k - 4, :]
                nc.vector.tensor_scalar_mul(hr, ones32_bf[:],
                                            hnew[:, k:k + 1])
            logits4 = cpool.tile([128, G_COLS], f32)
            se4 = wpool.tile([128, 1], f32, tag="se4")
            for cc in range(4):
                lg_ps = lg_pool.tile([128, CH], f32, tag="lg")
                for k in range(8):
                    for g in range(4):
                        c = 4 * cc + g
                        if c >= N_CH:
                            continue
                        csz = min(CH, V_SH - c * CH)
                        nc.tensor.matmul(lg_ps[32 * g:32 * g + 32, 0:csz],
                                         hrepA[:, k, :] if k < 4
                                         else hrepB[:, k - 4, :],
                                         wot_sb[:, k,
                                                c * CH:c * CH + csz],
                                         start=(k == 0), stop=(k == 7),
                                         skip_group_check=True,
                                         tile_position=(0, 32 * g))
                if cc < 3:
                    nc.vector.tensor_add(
                        logits4[:, cc * CH:(cc + 1) * CH],
                        lg_ps[:, 0:CH], bo4_sb[:, cc * CH:(cc + 1) * CH])
                else:
                    csz = V_SH - 12 * CH
                    nc.vector.tensor_add(
                        logits4[0:32, 3 * CH:3 * CH + csz],
                        lg_ps[0:32, 0:csz],
                        bo4_sb[0:32, 3 * CH:3 * CH + csz])

            # ---- dense exp + fused row-sums, then global AllGather ------
            etmp = cpool.tile([128, G_COLS], f32)
            se4b = wpool.tile([128, 1], f32, tag="se4b")
            nc.scalar.activation(etmp[:, 0:3 * CH], logits4[:, 0:3 * CH],
                                 AF.Exp, accum_out=se4[:])
            nc.scalar.activation(etmp[0:32, 3 * CH:G_COLS],
                                 logits4[0:32, 3 * CH:G_COLS],
                                 AF.Exp, accum_out=se4b[0:32, :])
            nc.vector.tensor_add(se4[0:1, :], se4[0:1, :], se4b[0:1, :])
            ccs_in = dpool.tile([1, 8], f32)
            ccs_out = dpool.tile([8, 8], f32)
            nc.gpsimd.dma_start(ccs_in[:], zz[:])
            nc.gpsimd.dma_start(
                ccs_in[0:1, 0:4],
                se4[:, :].rearrange("(a b) f -> a b f", b=32)[:, 0:1, :])
            nc.gpsimd.collective_compute(
                "AllGather", mybir.AluOpType.bypass, replica_groups=RG,
                ins=[ccs_in.opt()], outs=[ccs_out.opt()])
            s2row = wpool.tile([1, 32], f32, tag="s2row")
            nc.gpsimd.dma_start(s2row[:], ccs_out.opt()[:, 0:4])
            S2 = wpool.tile([1, 1], f32, tag="S2")
            nc.vector.reduce_sum(S2[:], s2row[:], axis=mybir.AxisListType.X)
            logS = wpool.tile([1, 1], f32, tag="logS")
            nc.scalar.activation(logS[:], S2[:], AF.Ln)
            ls_ps = sp_pool.tile([128, 1], f32, tag="sp")
            nc.tensor.matmul(ls_ps[:], aux_sb[0:1, 0:128], logS[:],
                             start=True, stop=True)
            logs128 = wpool.tile([128, 1], f32, tag="logs128")
            nc.scalar.copy(logs128[:], ls_ps[:])
            nlogs32 = wpool.tile([128, 1], f32, tag="nlogs32")
            nc.scalar.activation(nlogs32[0:32, :], logs128[0:32, :],
                                 AF.Copy, scale=-1.0)

            # out = logits - ln(S): two dense subtracts
            nc.vector.tensor_scalar_sub(logits4[:, 0:3 * CH],
                                        logits4[:, 0:3 * CH], logs128[:])
            nc.scalar.activation(logits4[0:32, 3 * CH:G_COLS],
                                 logits4[0:32, 3 * CH:G_COLS],
                                 AF.Identity, bias=nlogs32[0:32, :])

            # ---- output DMAs -------------------------------------------
            dst_all = out_sl.ap()[:, 0:4 * 3 * CH].rearrange(
                "p (cc g f) -> p g cc f", cc=3, g=4)
            src_all = logits4[:, 0:3 * CH].rearrange(
                "(a b) (cc f) -> a b cc f", b=32, f=CH)[:, 0:1, :, :]
            nc.sync.dma_start(dst_all, src_all)
            nc.sync.dma_start(out_sl.ap()[:, 12 * CH:V_SH],
                              logits4[0:1, 3 * CH:G_COLS])

            # ---- attention weights output (off the critical path) -------
            # w_o element d=128t+64e+j <- w_col[64e+j, t]
            wo_v = w_o.ap().rearrange("p (t e j) -> p e j t", e=2, j=64)
            nc.gpsimd.dma_start(wo_v[:, 0:1], w_col[0:64, 0:4])
            nc.gpsimd.dma_start(wo_v[:, 1:2], w_col[64:128, 0:4])

    nc.compile()
    return nc


def _col(v, ncols):
    return np.ascontiguousarray(v.reshape(ncols, 128).T)


def _prep_in_maps(inputs):
    f32 = np.float32
    x = np.asarray(inputs["x"]).reshape(-1)
    hidden = np.asarray(inputs["hidden"], f32).reshape(H)
    enc_full = np.ascontiguousarray(np.asarray(inputs["encoder_outputs"], f32))
    emb = np.asarray(inputs["emb"], f32)
    Ww = np.asarray(inputs["Ww"], f32)
    bw = np.asarray(inputs["bw"], f32)
    Wc = np.asarray(inputs["Wc"], f32)
    bc = np.asarray(inputs["bc"], f32)
    Wih = np.asarray(inputs["Wih"], f32)
    Whh = np.asarray(inputs["Whh"], f32)
    bih = np.asarray(inputs["bih"], f32)
    bhh = np.asarray(inputs["bhh"], f32)
    Wo = np.asarray(inputs["Wo"], f32)
    bo = np.asarray(inputs["bo"], f32)

    embed = emb[int(x[0])]
    cat1 = np.concatenate([embed, hidden])

    vecs = np.zeros((128, 68), f32)
    vecs[:, 0:16] = _col(cat1, 16)
    vecs[:, 16:24] = _col(embed, 8)
    vecs[:, 24:32] = _col(hidden, 8)
    vecs[:, 32] = 1.0
    vecs[:, 33:49] = _col((bih + bhh)[0:2048], 16)
    vecs[:, 49:57] = _col(bih[2048:], 8)
    vecs[:, 57:65] = _col(bhh[2048:], 8)

    aux = np.ones((1, 128), f32)

    pad = V_PAD - V
    Wo_pad = np.concatenate([Wo, np.zeros((pad, H), f32)], axis=0)
    bo_pad = np.concatenate([bo, np.full((pad,), -1e4, f32)])

    in_maps = []
    for r in range(NCORES):
        vr = vecs.copy()
        vr[0:64, 65] = bw[r * 64:(r + 1) * 64]
        vr[:, 66] = bc[r * 128:(r + 1) * 128]
        hs = slice(r * 128, (r + 1) * 128)
        bo_sh = bo_pad[r * V_SH:(r + 1) * V_SH]
        bo4m = np.zeros((4, G_COLS), f32)
        for c in range(N_CH):
            g, cc = c % 4, c // 4
            csz = min(CH, V_SH - c * CH)
            bo4m[g, cc * CH:cc * CH + csz] = bo_sh[c * CH:c * CH + csz]
        bo4m = np.repeat(bo4m, 32, axis=0)
        in_maps.append({
            "vecs": vr,
            "aux_row": aux,
            "wwt": np.ascontiguousarray(Ww[r * 64:(r + 1) * 64, :].T),
            "enc": enc_full.astype(_F16),
            "wct": np.ascontiguousarray(Wc[hs, :].T).astype(_F16),
            "wiht": np.ascontiguousarray(Wih[:, hs].T).astype(_F16),
            "whht": np.ascontiguousarray(Whh[:, hs].T).astype(_F16),
            "hloc": hidden[hs].reshape(128, 1).astype(_F16),
            "wot": np.ascontiguousarray(
                Wo_pad[r * V_SH:(r + 1) * V_SH, :].T).astype(_F16),
            "bo4": bo4m,
        })
    return in_maps


def _get_nc():
    if "nc" not in _cache:
        _cache["nc"] = _build()
    return _cache["nc"]


def _assemble(results):
    out = np.concatenate(
        [results[r]["out_sl"].reshape(-1) for r in range(NCORES)])[:V]
    out = np.ascontiguousarray(out.reshape(1, V), dtype=np.float32)
    h_new = np.ascontiguousarray(
        results[0]["hnew_o"].T.reshape(1, 1, H), dtype=np.float32)
    weights = np.ascontiguousarray(
        results[0]["w_o"].reshape(1, L), dtype=np.float32)
    return out, h_new, weights


def kernel(**inputs):
    from concourse.bass_utils import run_bass_kernel_spmd

    nc = _get_nc()
    in_maps = _prep_in_maps(inputs)
    res = run_bass_kernel_spmd(nc, in_maps, list(range(NCORES)))
    return _assemble(res.results)
